# revision 1
# baseline (speedup 1.0000x reference)
"""GAT (2-layer) Trainium2 kernel, SPMD across 8 NeuronCores.

Key algebra: segment softmax keyed by row is shift invariant, so the
(h[row] . a_l) term cancels and attention factorizes:
    alpha_e = g[col_e] * u[row_e],
    g[n] = exp(h[n] . a_r),   u[r] = 1 / sum_{e: row=r} g[col_e]
Each GAT layer then needs only two unweighted sparse ops over the fixed
graph:
    z   = A @ g          (segment-sum keyed by row)   -> u = 1/z
    agg = A^T @ (u * h)  (segment-sum keyed by col)
    out = g * agg
Both are done as: dma_gather of table rows per edge (128 edges/block) +
one-hot matmul (lhsT = one-hot of block-relative destination, built by a
DVE is_equal against an iota tile) accumulating into a PSUM window.

Sharding: z-phase edges by row range, aggregation edges by col range (each
core owns its 1250-node output slice). Cross-core: AllGather of u
([10000,H] f32) and of h1^T (5 MB) between the layers.

kernel(**inputs) takes FULL inputs and returns the FULL [10000, 22] output.
"""

import sys

sys.path.insert(0, "/opt/trn_rl_repo")

import numpy as np
import ml_dtypes

from concourse import bacc, mybir, tile
from concourse.bass_utils import run_bass_kernel_spmd

F32 = mybir.dt.float32
BF16 = mybir.dt.bfloat16
I16 = mybir.dt.int16
EXP = mybir.ActivationFunctionType.Exp
EQ = mybir.AluOpType.is_equal
MULT = mybir.AluOpType.mult
ADD = mybir.AluOpType.add
MIN = mybir.AluOpType.min
BYPASS = mybir.AluOpType.bypass

N = 10000
E = 320000
F = 128
H = 4
C = 22
P = 8
SLICE = N // P               # 1250 nodes per core
NWIN = (SLICE + 127) // 128  # 10 windows of <=128 dst/src nodes
NBLK = N // 128 + 1          # 79; always >= 1 pad block so row N is zero
NPAD = NBLK * 128            # 10112; table rows >= N are zero
OW1 = H * F                  # 512
CHUNK = 16                   # layer-1 gather chunk (128-edge blocks)
SKIP = set()                 # debug/timing: {"z", "agg1", "agg2"}


def _configure(n, e, p=8):
    """Shrink sizes for simulator debugging (same program structure)."""
    global N, E, P, SLICE, NWIN, NBLK, NPAD
    N, E, P = n, e, p
    SLICE = N // P
    NWIN = (SLICE + 127) // 128
    NBLK = N // 128 + 1
    NPAD = NBLK * 128


def _cdiv(a, b):
    return (a + b - 1) // b


def _wrap_idxs(idx):
    """dma_gather index layout: logical i at [i%16, i//16], replicated to
    128 partitions."""
    n = idx.shape[0]
    assert n % 16 == 0
    w = idx.reshape(n // 16, 16).T.astype(np.int16)
    return np.tile(w, (8, 1))


def _phase_arrays(key, other, nwin):
    """Group one core's (already core-local) edges by 128-wide key window.
    Returns per-window (rel, other) with rel = key - 128*w."""
    w = key >> 7
    order = np.argsort(w, kind="stable")
    key, other, w = key[order], other[order], w[order]
    out = []
    bounds = np.searchsorted(w, np.arange(nwin + 1))
    for i in range(nwin):
        sl = slice(bounds[i], bounds[i + 1])
        k, o = key[sl] - 128 * i, other[sl]
        so = np.argsort(o, kind="stable")  # sorted gather idx -> HBM locality
        out.append((k[so], o[so]))
    return out


def _build_edge_inputs(row, col):
    zraw, braw = [], []
    for k in range(P):
        base = k * SLICE
        m = (row >= base) & (row < base + SLICE)
        zraw.append(_phase_arrays(row[m] - base, col[m], NWIN))
        m = (col >= base) & (col < base + SLICE)
        braw.append(_phase_arrays(col[m] - base, row[m], NWIN))

    def block_counts(raw):
        return [
            max(_cdiv(max(max(len(raw[k][w][0]) for k in range(P)), 1), 128), 1)
            for w in range(NWIN)
        ]

    zB = block_counts(zraw)
    bB = block_counts(braw)

    def pack(raw, B):
        idx_l, rel_l = [], []
        for w in range(NWIN):
            n = B[w] * 128
            rel = np.zeros(n, np.int32)
            oth = np.full(n, N, np.int32)  # dummy -> zero table row
            r, o = raw[w]
            rel[: len(r)] = r
            oth[: len(o)] = o
            idx_l.append(_wrap_idxs(oth))
            rel_l.append(rel.reshape(B[w], 128).T.astype(np.float32))
        return np.concatenate(idx_l, 1), np.concatenate(rel_l, 1)

    per_core = []
    for k in range(P):
        zidx, zrel = pack(zraw[k], zB)
        bidx, brel = pack(braw[k], bB)
        base = k * SLICE
        gw = []
        for w in range(NWIN):
            nid = base + 128 * w + np.arange(128)
            nid = np.where(nid < base + SLICE, nid, N)
            gw.append(_wrap_idxs(nid))
        per_core.append(
            dict(
                zidx=zidx,
                zrel=zrel,
                bidx=bidx,
                brel_f=brel,
                gwidx=np.concatenate(gw, 1),
            )
        )
    return zB, bB, per_core


def _spmm(nc, tc, B, CH, idx_d, rel_d, tab, elem, rhs_w, psum_w, iof_t,
          name, flush, skip=False, bufs=3):
    """One-hot-matmul SpMM over 128-dst windows with gather chunks that span
    window boundaries. flush(w, po) consumes each window's PSUM result."""
    with (
        tc.tile_pool(name=f"gg{name}", bufs=bufs) as ggp,
        tc.tile_pool(name=f"gi{name}", bufs=bufs) as gip,
        tc.tile_pool(name=f"gr{name}", bufs=bufs) as grp,
        tc.tile_pool(name=f"go{name}", bufs=bufs) as ohp,
        tc.tile_pool(name=f"gp{name}", bufs=2, space="PSUM") as pp,
    ):
        total = sum(B)
        gts, ohs = {}, {}
        gb = 0
        for w, Bw in enumerate(B):
            po = pp.tile([128, psum_w], F32, tag="po")
            if skip:
                nc.vector.memset(po[:], 1.0)
                flush(w, po)
                continue
            for b in range(Bw):
                ch, off = divmod(gb, CH)
                if off == 0:
                    cb = min(CH, total - ch * CH)
                    it = gip.tile([128, CH * 8], I16, tag="gi")
                    nc.sync.dma_start(
                        it[:, : cb * 8],
                        idx_d[:, ch * CH * 8 : (ch * CH + cb) * 8],
                    )
                    gt = ggp.tile([128, CH, elem], BF16, tag="gg")
                    nc.gpsimd.dma_gather(
                        gt[:, :cb, :], tab[:], it[:, : cb * 8],
                        cb * 128, cb * 128, elem, single_packet=False,
                    )
                    rl = grp.tile([128, CH], F32, tag="gr")
                    nc.sync.dma_start(
                        rl[:, :cb], rel_d[:, ch * CH : ch * CH + cb]
                    )
                    oh = ohp.tile([128, CH, 128], BF16, tag="go")
                    nc.vector.tensor_tensor(
                        oh[:, :cb, :],
                        iof_t[:].rearrange("p (x f) -> p x f", x=1)
                        .broadcast_to([128, cb, 128]),
                        rl[:, :cb].rearrange("p (b x) -> p b x", x=1)
                        .broadcast_to([128, cb, 128]),
                        EQ,
                    )
                    gts[ch], ohs[ch] = gt, oh
                nc.tensor.matmul(
                    po[:], ohs[ch][:, off, :], gts[ch][:, off, 0:rhs_w],
                    start=(b == 0), stop=(b == Bw - 1),
                )
                gb += 1
            flush(w, po)


def _declare(nc, zB, bB):
    ZT, BT = sum(zB), sum(bB)
    T = type("T", (), {})()
    T.xT = nc.dram_tensor("xT", [F, NPAD], F32, kind="ExternalInput")
    T.W1 = nc.dram_tensor("W1", [F, OW1], F32, kind="ExternalInput")
    T.W2 = nc.dram_tensor("W2", [F, C], F32, kind="ExternalInput")
    T.a1rc = nc.dram_tensor("a1rc", [F, H], F32, kind="ExternalInput")
    T.a2rc = nc.dram_tensor("a2rc", [F, 1], F32, kind="ExternalInput")
    T.ident = nc.dram_tensor("ident", [128, 128], F32, kind="ExternalInput")
    T.iota_f = nc.dram_tensor("iota_f", [128, 128], F32, kind="ExternalInput")
    T.zidx_d = nc.dram_tensor("zidx", [128, ZT * 8], I16, kind="ExternalInput")
    T.zrel_d = nc.dram_tensor("zrel", [128, ZT], F32, kind="ExternalInput")
    T.bidx_d = nc.dram_tensor("bidx", [128, BT * 8], I16, kind="ExternalInput")
    T.brelf_d = nc.dram_tensor("brel_f", [128, BT], F32, kind="ExternalInput")
    T.gwidx_d = nc.dram_tensor("gwidx", [128, NWIN * 8], I16, kind="ExternalInput")
    T.out_d = nc.dram_tensor("out", [SLICE, C], F32, kind="ExternalOutput")

    T.g1_tab = nc.dram_tensor("g1_tab", [NPAD, 128], BF16)
    T.hh1_tab = nc.dram_tensor("hh1_tab", [NPAD, OW1], BF16)
    T.g2_tab = nc.dram_tensor("g2_tab", [NPAD, 128], BF16)
    T.hh2_tab = nc.dram_tensor("hh2_tab", [NPAD, 128], BF16)
    T.u1_sl = nc.dram_tensor("u1_sl", [SLICE, H], F32)
    T.u2_sl = nc.dram_tensor("u2_sl", [SLICE, 1], F32)
    T.u1_full = nc.dram_tensor("u1_full", [NPAD, H], F32, addr_space="Shared")
    T.u2_full = nc.dram_tensor("u2_full", [NPAD, 1], F32, addr_space="Shared")
    T.h1T_loc = nc.dram_tensor("h1T_loc", [F, SLICE], F32)
    T.h1T_ag = nc.dram_tensor("h1T_ag", [P, F, SLICE], F32, addr_space="Shared")

    return T


def _emit(nc, tc, T, zB, bB, s=""):
        groups = [list(range(P))]
        # ================= layer 1: dense + tables + z1 =================
        with (
            tc.tile_pool(name="persist" + s, bufs=1) as pp,
            tc.tile_pool(name="small" + s, bufs=3) as sp,
        ):
            W1_t = pp.tile([F, OW1], F32)
            nc.sync.dma_start(W1_t[:], T.W1[:])
            id_t = pp.tile([128, 128], F32)
            nc.sync.dma_start(id_t[:], T.ident[:])
            iof_t = pp.tile([128, 128], F32)
            nc.sync.dma_start(iof_t[:], T.iota_f[:])
            a1rc_t = pp.tile([F, H], F32)
            nc.sync.dma_start(a1rc_t[:], T.a1rc[:])
            W1ar_t = pp.tile([F, H], F32)

            with tc.tile_pool(name="ptr" + s, bufs=2, space="PSUM") as ptr:
                for hd in range(H):
                    pt = ptr.tile([128, 128], F32, tag="pt")
                    nc.tensor.transpose(pt[:], W1_t[:, hd * F : (hd + 1) * F], id_t[:])
                    w1t = sp.tile([128, 128], F32, tag="w1t")
                    nc.vector.tensor_copy(w1t[:], pt[:])
                    pv = ptr.tile([128, 1], F32, tag="pv")
                    nc.tensor.matmul(
                        pv[:], w1t[:], a1rc_t[:, hd : hd + 1], start=True, stop=True
                    )
                    nc.vector.tensor_copy(W1ar_t[:, hd : hd + 1], pv[:])

            h_nm = pp.tile([128, NBLK, OW1], F32)  # 20.2 MB
            g1_nm = pp.tile([128, NBLK, H], F32)
            with (
                tc.tile_pool(name="xtp" + s, bufs=3) as xtp,
                tc.tile_pool(name="ph" + s, bufs=2, space="PSUM") as php,
                tc.tile_pool(name="psr" + s, bufs=2, space="PSUM") as psrp,
            ):
                for b in range(NBLK):
                    xt = xtp.tile([128, 128], F32)
                    nc.sync.dma_start(xt[:], T.xT[:, b * 128 : (b + 1) * 128])
                    ph = php.tile([128, OW1], F32)
                    nc.tensor.matmul(ph[:], xt[:], W1_t[:], start=True, stop=True)
                    psr = psrp.tile([128, H], F32)
                    nc.tensor.matmul(psr[:], xt[:], W1ar_t[:], start=True, stop=True)
                    nc.vector.tensor_copy(h_nm[:, b, :], ph[:])
                    nc.scalar.activation(g1_nm[:, b, :], psr[:], EXP)

            with tc.tile_pool(name="stage" + s, bufs=1) as stp:
                st = stp.tile([128, NBLK, 128], BF16, tag="stage")
                nc.vector.memset(st[:], 0.0)
                nc.vector.tensor_copy(
                    st[:, : NBLK - 1, 0:H], g1_nm[:, : NBLK - 1, :]
                )
                nv = N - 128 * (NBLK - 1)
                if nv > 0:
                    nc.vector.tensor_copy(
                        st[0:nv, NBLK - 1, 0:H], g1_nm[0:nv, NBLK - 1, :]
                    )
                nc.sync.dma_start(
                    T.g1_tab.ap().rearrange("(b p) c -> p b c", p=128), st[:]
                )

            with tc.tile_pool(name="zu1" + s, bufs=3) as zup:

                def zflush1(w, po, zup=zup):
                    u_t = zup.tile([128, H], F32, tag="u")
                    nc.vector.reciprocal(u_t[:], po[:, 0:H])
                    rows = min(128, SLICE - 128 * w)
                    nc.sync.dma_start(
                        T.u1_sl[w * 128 : w * 128 + rows, :], u_t[0:rows, :]
                    )

                _spmm(nc, tc, zB, 32, T.zidx_d, T.zrel_d, T.g1_tab, 128, 8, 8,
                      iof_t, "z1" + s, zflush1, skip=("z" in SKIP), bufs=2)

            nc.gpsimd.collective_compute(
                "AllGather", BYPASS, groups,
                ins=[T.u1_sl[:].opt()], outs=[T.u1_full[0:N, :].opt()],
            )
            zt = sp.tile([NPAD - N, H], F32, tag="zpad")
            nc.vector.memset(zt[:], 0.0)
            nc.sync.dma_start(T.u1_full[N:NPAD, :], zt[:])

            u1_nm = pp.tile([128, NBLK, H], F32)
            nc.sync.dma_start(
                u1_nm[:], T.u1_full.ap().rearrange("(b p) c -> p b c", p=128)
            )
            with tc.tile_pool(name="hhp" + s, bufs=3) as hhp:
                for b in range(NBLK):
                    hh = hhp.tile([128, OW1], BF16)
                    for hd in range(H):
                        nc.vector.tensor_scalar(
                            hh[:, hd * F : (hd + 1) * F],
                            h_nm[:, b, hd * F : (hd + 1) * F],
                            u1_nm[:, b, hd : hd + 1],
                            None,
                            MULT,
                        )
                    nc.sync.dma_start(
                        T.hh1_tab.ap().rearrange("(b p) c -> p b c", p=128)[:, b, :],
                        hh[:],
                    )

        # ============ layer 1 aggregation + layer 2 (h_nm freed) ============
        with (
            tc.tile_pool(name="persist2" + s, bufs=1) as pp2,
            tc.tile_pool(name="small2" + s, bufs=3) as sp2,
        ):
            iof2 = pp2.tile([128, 128], F32)
            nc.sync.dma_start(iof2[:], T.iota_f[:])
            id2 = pp2.tile([128, 128], F32)
            nc.sync.dma_start(id2[:], T.ident[:])
            W2cat = pp2.tile([F, C + 1], F32)
            nc.sync.dma_start(W2cat[:, 0:C], T.W2[:])
            with tc.tile_pool(name="ptr2" + s, bufs=2, space="PSUM") as ptr:
                a2rc_t = sp2.tile([F, 1], F32, tag="T.a2rc")
                nc.sync.dma_start(a2rc_t[:], T.a2rc[:])
                pt = ptr.tile([128, 128], F32, tag="pt2")
                nc.tensor.transpose(pt[0:C, :], W2cat[:, 0:C], id2[:])
                w2t = sp2.tile([128, 128], F32, tag="w2t")
                nc.vector.tensor_copy(w2t[0:C, :], pt[0:C, :])
                pv = ptr.tile([128, 1], F32, tag="pv2")
                nc.tensor.matmul(
                    pv[:], w2t[0:C, :], a2rc_t[0:C, :], start=True, stop=True
                )
                nc.vector.tensor_copy(W2cat[:, C : C + 1], pv[:])

            h1T_sb = pp2.tile([128, NWIN * 128], F32)

            with (
                tc.tile_pool(name="gwp" + s, bufs=2) as gwp,
                tc.tile_pool(name="ptw" + s, bufs=2, space="PSUM") as ptw,
                tc.tile_pool(name="flush" + s, bufs=2) as flp,
            ):
                gwi = gwp.tile([128, NWIN * 8], I16, tag="gwi")
                nc.sync.dma_start(gwi[:], T.gwidx_d[:])
                gwb = gwp.tile([128, NWIN, 128], BF16, tag="gwb")
                nc.gpsimd.dma_gather(
                    gwb[:], T.g1_tab[:], gwi[:], NWIN * 128, NWIN * 128, 128,
                    single_packet=False,
                )
                gwf = gwp.tile([128, NWIN, 128], F32, tag="gwf")
                nc.vector.tensor_copy(gwf[:], gwb[:])

                def flush1(w, po):
                    o_t = flp.tile([128, OW1], F32, tag="o")
                    for hd in range(H):
                        nc.vector.tensor_scalar(
                            o_t[:, hd * F : (hd + 1) * F],
                            po[:, hd * F : (hd + 1) * F],
                            gwf[:, w, hd : hd + 1],
                            None, MULT,
                        )
                    # elu(x) = relu(x) + exp(min(x,0)) - 1 ; h1 = mean_heads
                    neg = flp.tile([128, OW1], F32, tag="neg")
                    nc.vector.tensor_scalar(neg[:], o_t[:], 0.0, None, MIN)
                    ex = flp.tile([128, OW1], F32, tag="ex")
                    nc.scalar.activation(ex[:], neg[:], EXP)
                    rl = flp.tile([128, OW1], F32, tag="rl")
                    nc.vector.tensor_relu(rl[:], o_t[:])
                    su = flp.tile([128, OW1], F32, tag="su")
                    nc.vector.tensor_tensor(su[:], rl[:], ex[:], ADD)
                    t01 = flp.tile([128, F], F32, tag="t01")
                    nc.vector.tensor_tensor(t01[:], su[:, 0:F], su[:, F : 2 * F], ADD)
                    t23 = flp.tile([128, F], F32, tag="t23")
                    nc.vector.tensor_tensor(
                        t23[:], su[:, 2 * F : 3 * F], su[:, 3 * F :], ADD
                    )
                    h1_t = flp.tile([128, F], F32, tag="h1")
                    nc.vector.tensor_tensor(h1_t[:], t01[:], t23[:], ADD)
                    nc.vector.tensor_scalar(h1_t[:], h1_t[:], 0.25, -1.0, MULT, ADD)
                    ptt = ptw.tile([128, 128], F32, tag="ptt")
                    nc.tensor.transpose(ptt[:], h1_t[:], id2[:])
                    nc.vector.tensor_copy(h1T_sb[:, w * 128 : (w + 1) * 128], ptt[:])

                _spmm(nc, tc, bB, CHUNK, T.bidx_d, T.brelf_d, T.hh1_tab, OW1,
                      OW1, OW1, iof2, "a1" + s, flush1, skip=("agg1" in SKIP),
                      bufs=3)

            nc.sync.dma_start(T.h1T_loc[:], h1T_sb[:, 0:SLICE])
            nc.gpsimd.collective_compute(
                "AllGather", BYPASS, groups,
                ins=[T.h1T_loc[:].opt()], outs=[T.h1T_ag[:].opt()],
            )
            h1T_full = pp2.tile([128, P, SLICE], F32)
            nc.sync.dma_start(h1T_full[:], T.h1T_ag.ap().rearrange("s f n -> f s n"))
            h1T_flat = h1T_full[:].rearrange("f s n -> f (s n)")

            h2_nm = pp2.tile([128, NBLK, C], F32)
            g2_nm = pp2.tile([128, NBLK, 1], F32)
            with tc.tile_pool(name="ph2" + s, bufs=2, space="PSUM") as ph2p:
                for b in range(NBLK):
                    nv = max(0, min(128, N - b * 128))
                    if nv < 128:
                        nc.vector.memset(h2_nm[:, b, :], 0.0)
                        nc.vector.memset(g2_nm[:, b, :], 0.0)
                    if nv == 0:
                        continue
                    ph2 = ph2p.tile([128, C + 1], F32)
                    nc.tensor.matmul(
                        ph2[0:nv, :],
                        h1T_flat[:, b * 128 : b * 128 + nv],
                        W2cat[:],
                        start=True,
                        stop=True,
                    )
                    nc.vector.tensor_copy(h2_nm[0:nv, b, :], ph2[0:nv, 0:C])
                    nc.scalar.activation(g2_nm[0:nv, b, :], ph2[0:nv, C : C + 1], EXP)

            with tc.tile_pool(name="stage2" + s, bufs=1) as stp:
                st = stp.tile([128, NBLK, 128], BF16, tag="stage2")
                nc.vector.memset(st[:], 0.0)
                nc.vector.tensor_copy(st[:, :, 0:1], g2_nm[:])
                nc.sync.dma_start(
                    T.g2_tab.ap().rearrange("(b p) c -> p b c", p=128), st[:]
                )

            with tc.tile_pool(name="zu2" + s, bufs=3) as zup:

                def zflush2(w, po, zup=zup):
                    u_t = zup.tile([128, 1], F32, tag="u2")
                    nc.vector.reciprocal(u_t[:], po[:, 0:1])
                    rows = min(128, SLICE - 128 * w)
                    nc.sync.dma_start(
                        T.u2_sl[w * 128 : w * 128 + rows, :], u_t[0:rows, :]
                    )

                _spmm(nc, tc, zB, 32, T.zidx_d, T.zrel_d, T.g2_tab, 128, 8, 8,
                      iof2, "z2" + s, zflush2, skip=("z" in SKIP), bufs=3)

            nc.gpsimd.collective_compute(
                "AllGather", BYPASS, groups,
                ins=[T.u2_sl[:].opt()], outs=[T.u2_full[0:N, :].opt()],
            )
            zt2 = sp2.tile([NPAD - N, 1], F32, tag="zpad2")
            nc.vector.memset(zt2[:], 0.0)
            nc.sync.dma_start(T.u2_full[N:NPAD, :], zt2[:])

            u2_nm = pp2.tile([128, NBLK, 1], F32)
            nc.sync.dma_start(
                u2_nm[:], T.u2_full.ap().rearrange("(b p) c -> p b c", p=128)
            )
            with tc.tile_pool(name="stage3" + s, bufs=1) as stp:
                st = stp.tile([128, NBLK, 128], BF16, tag="stage3")
                nc.vector.memset(st[:], 0.0)
                for b in range(NBLK):
                    nc.vector.tensor_scalar(
                        st[:, b, 0:C], h2_nm[:, b, :], u2_nm[:, b, :], None, MULT
                    )
                nc.sync.dma_start(
                    T.hh2_tab.ap().rearrange("(b p) c -> p b c", p=128), st[:]
                )

            with (
                tc.tile_pool(name="gw2" + s, bufs=2) as gwp,
                tc.tile_pool(name="fl2" + s, bufs=2) as flp,
            ):
                gwi = gwp.tile([128, NWIN * 8], I16, tag="gwi2")
                nc.sync.dma_start(gwi[:], T.gwidx_d[:])
                gwb = gwp.tile([128, NWIN, 128], BF16, tag="gwb2")
                nc.gpsimd.dma_gather(
                    gwb[:], T.g2_tab[:], gwi[:], NWIN * 128, NWIN * 128, 128,
                    single_packet=False,
                )
                gwf = gwp.tile([128, NWIN, 128], F32, tag="gwf2")
                nc.vector.tensor_copy(gwf[:], gwb[:])

                def flush2(w, po):
                    o2 = flp.tile([128, C], F32, tag="o2")
                    nc.vector.tensor_scalar(
                        o2[:], po[:, 0:C], gwf[:, w, 0:1], None, MULT
                    )
                    rows = min(128, SLICE - 128 * w)
                    nc.sync.dma_start(
                        T.out_d[w * 128 : w * 128 + rows, :], o2[0:rows, :]
                    )

                _spmm(nc, tc, bB, 32, T.bidx_d, T.brelf_d, T.hh2_tab, 128,
                      C, C, iof2, "a2" + s, flush2, skip=("agg2" in SKIP),
                      bufs=3)



def _build_program(zB, bB, reps=1):
    nc = bacc.Bacc("TRN2", target_bir_lowering=False, debug=False, num_devices=P)
    groups = [list(range(P))]
    T = _declare(nc, zB, bB)
    with tile.TileContext(nc) as tc:
        for r in range(reps):
            _emit(nc, tc, T, zB, bB, s=str(r))
            if reps > 1:
                with tc.tile_critical():
                    nc.all_core_barrier()
    nc.compile()
    return nc


def _host_inputs(x, W1, a1, W2, a2):
    xT = np.zeros((F, NPAD), np.float32)
    xT[:, :N] = np.ascontiguousarray(np.asarray(x, np.float32).T)
    a1 = np.asarray(a1, np.float32)
    a2 = np.asarray(a2, np.float32)
    a1rc = np.ascontiguousarray(a1[:, F : 2 * F].T)  # [128, H]
    a2rc = np.zeros((F, 1), np.float32)
    a2rc[0:C, 0] = a2[0, C : 2 * C]
    iota = np.tile(np.arange(128, dtype=np.float32), (128, 1))
    return dict(
        xT=xT,
        W1=np.asarray(W1, np.float32),
        W2=np.asarray(W2, np.float32),
        a1rc=a1rc,
        a2rc=a2rc,
        ident=np.eye(128, dtype=np.float32),
        iota_f=np.ascontiguousarray(iota),
    )


def build(x, edge_index, W1, a1, W2, a2, reps=1):
    """Build program + per-core input maps. Returns (nc, in_maps)."""
    ei = np.asarray(edge_index)
    row = ei[0].astype(np.int64)
    col = ei[1].astype(np.int64)
    zB, bB, per_core = _build_edge_inputs(row, col)
    nc = _build_program(zB, bB, reps=reps)
    common = _host_inputs(x, W1, a1, W2, a2)
    in_maps = [{**common, **per_core[k]} for k in range(P)]
    return nc, in_maps


def kernel(x, edge_index, W1, a1, W2, a2):
    nc, in_maps = build(x, edge_index, W1, a1, W2, a2)
    res = run_bass_kernel_spmd(nc, in_maps, list(range(P)))
    return np.concatenate([res.results[k]["out"] for k in range(P)], axis=0)



# revision 2
# speedup vs baseline: 8.4045x; 8.4045x over previous
"""GAT (2-layer) Trainium2 kernel, SPMD across 8 NeuronCores — v2.

Same device algorithm as the baseline (factorized segment softmax +
one-hot-matmul SpMM over dma_gathered table rows), but optimized for
end-to-end wall time per run, which is dominated by host->device input
transfer and per-call jit compilation, not HW execution:

  * ONE packed uint8 input tensor per core (~0.7 MB vs 7.3 MB over 12
    tensors). Sections (per-partition layout, bitcast views on device):
      x shard   f16 [128, XS]    node features, transposed + sharded;
                                 AllGathered on-device across the 8 cores
      W1        f16 [128, 512]
      params    f16 [128, 28]    W2 | a1rc | a2rc
      rel       u8  [128, ZT+BT] block-relative dst indices (0..127)
      idx       i16 [16, WI] wrapped gather indices, stored across 128
                                 partitions; replicated 16->128 on device
  * identity / iota helper matrices generated on device (iota + is_equal)
  * f16 everywhere the baseline used bf16 (same bytes, 8x less rounding)
  * jax persistent compilation cache enabled so repeated
    run_bass_kernel_spmd calls skip the ~1s per-call PJRT re-compile.

kernel(**inputs) takes FULL inputs and returns the FULL [10000, 22] output.
"""

import sys

sys.path.insert(0, "/opt/trn_rl_repo")

import numpy as np

import jax

jax.config.update("jax_compilation_cache_dir", "/tmp/jax_bass_cache")
jax.config.update("jax_persistent_cache_min_entry_size_bytes", -1)
jax.config.update("jax_persistent_cache_min_compile_time_secs", 0)

from concourse import bacc, mybir, tile
from concourse.bass_utils import run_bass_kernel_spmd

F32 = mybir.dt.float32
F16 = mybir.dt.float16
I16 = mybir.dt.int16
I32 = mybir.dt.int32
U8 = mybir.dt.uint8
EXP = mybir.ActivationFunctionType.Exp
EQ = mybir.AluOpType.is_equal
MULT = mybir.AluOpType.mult
ADD = mybir.AluOpType.add
MIN = mybir.AluOpType.min
BYPASS = mybir.AluOpType.bypass

N = 10000
E = 320000
F = 128
H = 4
C = 22
P = 8
SLICE = N // P               # 1250 nodes per core
NWIN = (SLICE + 127) // 128  # 10 windows of <=128 dst/src nodes
NBLK = N // 128 + 1          # 79; always >= 1 pad block so row N is zero
NPAD = NBLK * 128            # 10112; table rows >= N are zero
XBLK = (NBLK + P - 1) // P   # x-shard blocks per core
XS = XBLK * 128              # x-shard width (1280)
NPADX = P * XS               # 10240
OW1 = H * F                  # 512
CHUNK = 16                   # layer-1 gather chunk (128-edge blocks)
SKIP = set()                 # debug/timing: {"z", "agg1", "agg2"}


def _configure(n, e, p=8):
    """Shrink sizes for simulator debugging (same program structure)."""
    global N, E, P, SLICE, NWIN, NBLK, NPAD, XBLK, XS, NPADX
    N, E, P = n, e, p
    SLICE = N // P
    NWIN = (SLICE + 127) // 128
    NBLK = N // 128 + 1
    NPAD = NBLK * 128
    XBLK = (NBLK + P - 1) // P
    XS = XBLK * 128
    NPADX = P * XS


def _cdiv(a, b):
    return (a + b - 1) // b


def _wrap16(idx):
    """dma_gather index layout: logical i at [i%16, i//16] (16 partitions,
    NOT replicated — replication to 128 partitions happens on device)."""
    n = idx.shape[0]
    assert n % 16 == 0
    return idx.reshape(n // 16, 16).T.astype(np.int16)


def _phase_arrays(key, other, nwin):
    """Group one core's (already core-local) edges by 128-wide key window.
    Returns per-window (rel, other) with rel = key - 128*w."""
    w = key >> 7
    order = np.argsort(w, kind="stable")
    key, other, w = key[order], other[order], w[order]
    out = []
    bounds = np.searchsorted(w, np.arange(nwin + 1))
    for i in range(nwin):
        sl = slice(bounds[i], bounds[i + 1])
        k, o = key[sl] - 128 * i, other[sl]
        so = np.argsort(o, kind="stable")  # sorted gather idx -> HBM locality
        out.append((k[so], o[so]))
    return out


def _build_edge_inputs(row, col):
    zraw, braw = [], []
    for k in range(P):
        base = k * SLICE
        m = (row >= base) & (row < base + SLICE)
        zraw.append(_phase_arrays(row[m] - base, col[m], NWIN))
        m = (col >= base) & (col < base + SLICE)
        braw.append(_phase_arrays(col[m] - base, row[m], NWIN))

    def block_counts(raw):
        return [
            max(_cdiv(max(max(len(raw[k][w][0]) for k in range(P)), 1), 128), 1)
            for w in range(NWIN)
        ]

    zB = block_counts(zraw)
    bB = block_counts(braw)

    def pack(raw, B):
        idx_l, rel_l = [], []
        for w in range(NWIN):
            n = B[w] * 128
            rel = np.zeros(n, np.uint8)
            oth = np.full(n, N, np.int32)  # dummy -> zero table row
            r, o = raw[w]
            rel[: len(r)] = r
            oth[: len(o)] = o
            idx_l.append(_wrap16(oth))
            rel_l.append(rel.reshape(B[w], 128).T)
        return np.concatenate(idx_l, 1), np.concatenate(rel_l, 1)

    per_core = []
    for k in range(P):
        zidx, zrel = pack(zraw[k], zB)
        bidx, brel = pack(braw[k], bB)
        base = k * SLICE
        gw = []
        for w in range(NWIN):
            nid = base + 128 * w + np.arange(128)
            nid = np.where(nid < base + SLICE, nid, N)
            gw.append(_wrap16(nid))
        per_core.append(
            dict(zidx=zidx, zrel=zrel, bidx=bidx, brel=brel,
                 gwidx=np.concatenate(gw, 1))
        )
    return zB, bB, per_core


# ---------------------------------------------------------------------------
# blob layout (per-partition byte offsets)


def _blob_layout(ZT, BT):
    WI8 = ZT + BT + NWIN              # i16 per partition of idx section
    o = {}
    o["x"] = 0
    o["w1"] = o["x"] + 2 * XS
    o["prm"] = o["w1"] + 2 * OW1
    o["rel"] = o["prm"] + 2 * 28
    oidx = o["rel"] + (ZT + BT)
    o["idx"] = oidx + (oidx % 2)      # 2-byte align for i16 bitcast
    o["end"] = o["idx"] + 2 * WI8
    o["WI8"] = WI8
    return o


def _spmm(nc, tc, B, CH, idx128, idx_base, rel_sb, rel_base, tab, elem,
          rhs_w, psum_w, iof_t, name, flush, skip=False, bufs=3):
    """One-hot-matmul SpMM over 128-dst windows with gather chunks that span
    window boundaries. flush(w, po) consumes each window's PSUM result.
    idx_base/rel_base are in 128-edge-block units into idx128/rel_sb."""
    with (
        tc.tile_pool(name=f"gg{name}", bufs=bufs) as ggp,
        tc.tile_pool(name=f"gi{name}", bufs=bufs) as gip,
        tc.tile_pool(name=f"go{name}", bufs=bufs) as ohp,
        tc.tile_pool(name=f"gp{name}", bufs=2, space="PSUM") as pp,
    ):
        total = sum(B)
        gts, ohs = {}, {}
        gb = 0
        for w, Bw in enumerate(B):
            po = pp.tile([128, psum_w], F32, tag="po")
            if skip:
                nc.vector.memset(po[:], 1.0)
                flush(w, po)
                continue
            for b in range(Bw):
                ch, off = divmod(gb, CH)
                if off == 0:
                    cb = min(CH, total - ch * CH)
                    it = gip.tile([128, CH * 8], I16, tag="gi")
                    nc.sync.dma_start(
                        it[:, : cb * 8],
                        idx128[:, (idx_base + ch * CH) * 8
                               : (idx_base + ch * CH + cb) * 8],
                    )
                    gt = ggp.tile([128, CH, elem], F16, tag="gg")
                    nc.gpsimd.dma_gather(
                        gt[:, :cb, :], tab[:], it[:, : cb * 8],
                        cb * 128, cb * 128, elem, single_packet=False,
                    )
                    oh = ohp.tile([128, CH, 128], F16, tag="go")
                    nc.vector.tensor_tensor(
                        oh[:, :cb, :],
                        iof_t[:].rearrange("p (x f) -> p x f", x=1)
                        .broadcast_to([128, cb, 128]),
                        rel_sb[:, rel_base + ch * CH : rel_base + ch * CH + cb]
                        .rearrange("p (b x) -> p b x", x=1)
                        .broadcast_to([128, cb, 128]),
                        EQ,
                    )
                    gts[ch], ohs[ch] = gt, oh
                nc.tensor.matmul(
                    po[:], ohs[ch][:, off, :], gts[ch][:, off, 0:rhs_w],
                    start=(b == 0), stop=(b == Bw - 1),
                )
                gb += 1
            flush(w, po)


def _declare(nc, zB, bB):
    ZT, BT = sum(zB), sum(bB)
    L = _blob_layout(ZT, BT)
    T = type("T", (), {})()
    T.L = L
    T.blob = nc.dram_tensor("blob", [128, L["end"]], U8, kind="ExternalInput")
    T.out_d = nc.dram_tensor("out", [SLICE, C], F32, kind="ExternalOutput")

    T.x_sl = nc.dram_tensor("x_sl", [128, XS], F16)
    T.x_ag = nc.dram_tensor("x_ag", [P, 128, XS], F16, addr_space="Shared")
    T.idx128 = nc.dram_tensor("idx128", [128, L["WI8"] * 8], I16)

    T.g1_tab = nc.dram_tensor("g1_tab", [NPAD, 128], F16)
    T.hh1_tab = nc.dram_tensor("hh1_tab", [NPAD, OW1], F16)
    T.g2_tab = nc.dram_tensor("g2_tab", [NPAD, 128], F16)
    T.hh2_tab = nc.dram_tensor("hh2_tab", [NPAD, 128], F16)
    T.u1_sl = nc.dram_tensor("u1_sl", [SLICE, H], F32)
    T.u2_sl = nc.dram_tensor("u2_sl", [SLICE, 1], F32)
    T.u1_full = nc.dram_tensor("u1_full", [NPAD, H], F32, addr_space="Shared")
    T.u2_full = nc.dram_tensor("u2_full", [NPAD, 1], F32, addr_space="Shared")
    T.h1T_loc = nc.dram_tensor("h1T_loc", [F, SLICE], F32)
    T.h1T_ag = nc.dram_tensor("h1T_ag", [P, F, SLICE], F32, addr_space="Shared")

    return T


def _emit(nc, tc, T, zB, bB, s=""):
        groups = [list(range(P))]
        L = T.L
        ZT, BT = sum(zB), sum(bB)
        blob = T.blob.ap()
        xv = blob[:, L["x"] : L["x"] + 2 * XS].bitcast(F16)
        w1v = blob[:, L["w1"] : L["w1"] + 2 * OW1].bitcast(F16)
        prmv = blob[:, L["prm"] : L["prm"] + 2 * 28].bitcast(F16)
        relv = blob[:, L["rel"] : L["rel"] + ZT + BT]
        idxv = (
            blob[:, L["idx"] : L["idx"] + 2 * L["WI8"]]
            .bitcast(I16)
            .rearrange("(a b) c -> a b c", a=16)
        )  # [16, 8, WI8]; element (a,b,c) = W16[a, b*WI8+c]
        RELW = ZT + BT

        # ================= layer 1: dense + tables + z1 =================
        with (
            tc.tile_pool(name="persist" + s, bufs=1) as pp,
            tc.tile_pool(name="small" + s, bufs=3) as sp,
        ):
            # ---- unpack blob: x shard -> AllGather; idx 16->128 ----
            nc.sync.dma_start(T.x_sl[:], xv)
            nc.gpsimd.collective_compute(
                "AllGather", BYPASS, groups,
                ins=[T.x_sl[:].opt()], outs=[T.x_ag[:].opt()],
            )
            for r in range(8):
                nc.sync.dma_start(
                    T.idx128[16 * r : 16 * r + 16, :]
                    .rearrange("a (b c) -> a b c", b=8),
                    idxv,
                )

            # ---- helper matrices generated on device ----
            it32 = sp.tile([128, 128], I32, tag="it32")
            nc.gpsimd.iota(it32[:], [[1, 128]], base=0, channel_multiplier=0)
            ip32 = sp.tile([128, 128], I32, tag="ip32")
            nc.gpsimd.iota(ip32[:], [[0, 128]], base=0, channel_multiplier=1)
            iof_t = pp.tile([128, 128], F32)
            nc.vector.tensor_copy(iof_t[:], it32[:])
            id16 = pp.tile([128, 128], F16)
            nc.vector.tensor_tensor(id16[:], it32[:], ip32[:], EQ)
            id32 = pp.tile([128, 128], F32)
            nc.vector.tensor_tensor(id32[:], it32[:], ip32[:], EQ)

            # ---- params / weights / rel ----
            W1_t = pp.tile([F, OW1], F16)
            nc.sync.dma_start(W1_t[:], w1v)
            prm_t = pp.tile([128, 28], F16)
            nc.sync.dma_start(prm_t[:], prmv)
            rel8 = sp.tile([128, RELW], U8, tag="rel8")
            nc.sync.dma_start(rel8[:], relv)
            rel_sb = pp.tile([128, RELW], F32)
            nc.vector.tensor_copy(rel_sb[:], rel8[:])

            W1ar_t = pp.tile([F, H], F16)
            with tc.tile_pool(name="ptr" + s, bufs=2, space="PSUM") as ptr:
                for hd in range(H):
                    pt = ptr.tile([128, 128], F16, tag="pt")
                    nc.tensor.transpose(pt[:], W1_t[:, hd * F : (hd + 1) * F], id16[:])
                    w1t = sp.tile([128, 128], F16, tag="w1t")
                    nc.vector.tensor_copy(w1t[:], pt[:])
                    pv = ptr.tile([128, 1], F32, tag="pv")
                    nc.tensor.matmul(
                        pv[:], w1t[:], prm_t[:, 22 + hd : 23 + hd],
                        start=True, stop=True,
                    )
                    nc.vector.tensor_copy(W1ar_t[:, hd : hd + 1], pv[:])

            xfull = pp.tile([128, P, XS], F16)  # full gathered x^T in SBUF
            nc.sync.dma_start(xfull[:], T.x_ag.ap().rearrange("p f s -> f p s"))

            h_nm = pp.tile([128, NBLK, OW1], F16)  # 10.1 MB
            g1_nm = pp.tile([128, NBLK, H], F32)
            with (
                tc.tile_pool(name="ph" + s, bufs=2, space="PSUM") as php,
                tc.tile_pool(name="psr" + s, bufs=2, space="PSUM") as psrp,
            ):
                for b in range(NBLK):
                    xt = xfull[:, b // XBLK, (b % XBLK) * 128 : (b % XBLK + 1) * 128]
                    ph = php.tile([128, OW1], F32)
                    nc.tensor.matmul(ph[:], xt, W1_t[:], start=True, stop=True)
                    psr = psrp.tile([128, H], F32)
                    nc.tensor.matmul(psr[:], xt, W1ar_t[:], start=True, stop=True)
                    nc.vector.tensor_copy(h_nm[:, b, :], ph[:])
                    nc.scalar.activation(g1_nm[:, b, :], psr[:], EXP)

            with tc.tile_pool(name="stage" + s, bufs=1) as stp:
                st = stp.tile([128, NBLK, 128], F16, tag="stage")
                nc.vector.memset(st[:], 0.0)
                nc.vector.tensor_copy(
                    st[:, : NBLK - 1, 0:H], g1_nm[:, : NBLK - 1, :]
                )
                nv = N - 128 * (NBLK - 1)
                if nv > 0:
                    nc.vector.tensor_copy(
                        st[0:nv, NBLK - 1, 0:H], g1_nm[0:nv, NBLK - 1, :]
                    )
                nc.sync.dma_start(
                    T.g1_tab.ap().rearrange("(b p) c -> p b c", p=128), st[:]
                )

            with tc.tile_pool(name="zu1" + s, bufs=3) as zup:

                def zflush1(w, po, zup=zup):
                    u_t = zup.tile([128, H], F32, tag="u")
                    nc.vector.reciprocal(u_t[:], po[:, 0:H])
                    rows = min(128, SLICE - 128 * w)
                    nc.sync.dma_start(
                        T.u1_sl[w * 128 : w * 128 + rows, :], u_t[0:rows, :]
                    )

                _spmm(nc, tc, zB, 32, T.idx128, 0, rel_sb, 0, T.g1_tab, 128,
                      8, 8, iof_t, "z1" + s, zflush1, skip=("z" in SKIP),
                      bufs=2)

            nc.gpsimd.collective_compute(
                "AllGather", BYPASS, groups,
                ins=[T.u1_sl[:].opt()], outs=[T.u1_full[0:N, :].opt()],
            )
            zt = sp.tile([NPAD - N, H], F32, tag="zpad")
            nc.vector.memset(zt[:], 0.0)
            nc.sync.dma_start(T.u1_full[N:NPAD, :], zt[:])

            u1_nm = pp.tile([128, NBLK, H], F32)
            nc.sync.dma_start(
                u1_nm[:], T.u1_full.ap().rearrange("(b p) c -> p b c", p=128)
            )
            with tc.tile_pool(name="hhp" + s, bufs=3) as hhp:
                for b in range(NBLK):
                    hh = hhp.tile([128, OW1], F16)
                    for hd in range(H):
                        nc.vector.tensor_scalar(
                            hh[:, hd * F : (hd + 1) * F],
                            h_nm[:, b, hd * F : (hd + 1) * F],
                            u1_nm[:, b, hd : hd + 1],
                            None,
                            MULT,
                        )
                    nc.sync.dma_start(
                        T.hh1_tab.ap().rearrange("(b p) c -> p b c", p=128)[:, b, :],
                        hh[:],
                    )

        # ============ layer 1 aggregation + layer 2 (h_nm freed) ============
        with (
            tc.tile_pool(name="persist2" + s, bufs=1) as pp2,
            tc.tile_pool(name="small2" + s, bufs=3) as sp2,
        ):
            ZT, BT = sum(zB), sum(bB)
            it32b = sp2.tile([128, 128], I32, tag="it32b")
            nc.gpsimd.iota(it32b[:], [[1, 128]], base=0, channel_multiplier=0)
            ip32b = sp2.tile([128, 128], I32, tag="ip32b")
            nc.gpsimd.iota(ip32b[:], [[0, 128]], base=0, channel_multiplier=1)
            iof2 = pp2.tile([128, 128], F32)
            nc.vector.tensor_copy(iof2[:], it32b[:])
            id2 = pp2.tile([128, 128], F32)
            nc.vector.tensor_tensor(id2[:], it32b[:], ip32b[:], EQ)
            prm2_t = pp2.tile([128, 28], F16)
            nc.sync.dma_start(prm2_t[:], prmv)
            rel8b = sp2.tile([128, RELW], U8, tag="rel8b")
            nc.sync.dma_start(rel8b[:], relv)
            rel_sb2 = pp2.tile([128, RELW], F32)
            nc.vector.tensor_copy(rel_sb2[:], rel8b[:])

            W2cat = pp2.tile([F, C + 1], F32)
            nc.vector.tensor_copy(W2cat[:, 0:C], prm2_t[:, 0:22])
            with tc.tile_pool(name="ptr2" + s, bufs=2, space="PSUM") as ptr:
                a2rc_t = sp2.tile([F, 1], F32, tag="a2rc")
                nc.vector.tensor_copy(a2rc_t[:], prm2_t[:, 26:27])
                pt = ptr.tile([128, 128], F32, tag="pt2")
                nc.tensor.transpose(pt[0:C, :], W2cat[:, 0:C], id2[:])
                w2t = sp2.tile([128, 128], F32, tag="w2t")
                nc.vector.tensor_copy(w2t[0:C, :], pt[0:C, :])
                pv = ptr.tile([128, 1], F32, tag="pv2")
                nc.tensor.matmul(
                    pv[:], w2t[0:C, :], a2rc_t[0:C, :], start=True, stop=True
                )
                nc.vector.tensor_copy(W2cat[:, C : C + 1], pv[:])

            h1T_sb = pp2.tile([128, NWIN * 128], F32)

            with (
                tc.tile_pool(name="gwp" + s, bufs=2) as gwp,
                tc.tile_pool(name="ptw" + s, bufs=2, space="PSUM") as ptw,
                tc.tile_pool(name="flush" + s, bufs=2) as flp,
            ):
                gwi = gwp.tile([128, NWIN * 8], I16, tag="gwi")
                nc.sync.dma_start(
                    gwi[:], T.idx128[:, (ZT + BT) * 8 : (ZT + BT + NWIN) * 8]
                )
                gwb = gwp.tile([128, NWIN, 128], F16, tag="gwb")
                nc.gpsimd.dma_gather(
                    gwb[:], T.g1_tab[:], gwi[:], NWIN * 128, NWIN * 128, 128,
                    single_packet=False,
                )
                gwf = gwp.tile([128, NWIN, 128], F32, tag="gwf")
                nc.vector.tensor_copy(gwf[:], gwb[:])

                def flush1(w, po):
                    o_t = flp.tile([128, OW1], F32, tag="o")
                    for hd in range(H):
                        nc.vector.tensor_scalar(
                            o_t[:, hd * F : (hd + 1) * F],
                            po[:, hd * F : (hd + 1) * F],
                            gwf[:, w, hd : hd + 1],
                            None, MULT,
                        )
                    # elu(x) = relu(x) + exp(min(x,0)) - 1 ; h1 = mean_heads
                    neg = flp.tile([128, OW1], F32, tag="neg")
                    nc.vector.tensor_scalar(neg[:], o_t[:], 0.0, None, MIN)
                    ex = flp.tile([128, OW1], F32, tag="ex")
                    nc.scalar.activation(ex[:], neg[:], EXP)
                    rl = flp.tile([128, OW1], F32, tag="rl")
                    nc.vector.tensor_relu(rl[:], o_t[:])
                    su = flp.tile([128, OW1], F32, tag="su")
                    nc.vector.tensor_tensor(su[:], rl[:], ex[:], ADD)
                    t01 = flp.tile([128, F], F32, tag="t01")
                    nc.vector.tensor_tensor(t01[:], su[:, 0:F], su[:, F : 2 * F], ADD)
                    t23 = flp.tile([128, F], F32, tag="t23")
                    nc.vector.tensor_tensor(
                        t23[:], su[:, 2 * F : 3 * F], su[:, 3 * F :], ADD
                    )
                    h1_t = flp.tile([128, F], F32, tag="h1")
                    nc.vector.tensor_tensor(h1_t[:], t01[:], t23[:], ADD)
                    nc.vector.tensor_scalar(h1_t[:], h1_t[:], 0.25, -1.0, MULT, ADD)
                    ptt = ptw.tile([128, 128], F32, tag="ptt")
                    nc.tensor.transpose(ptt[:], h1_t[:], id2[:])
                    nc.vector.tensor_copy(h1T_sb[:, w * 128 : (w + 1) * 128], ptt[:])

                _spmm(nc, tc, bB, CHUNK, T.idx128, ZT, rel_sb2, ZT,
                      T.hh1_tab, OW1, OW1, OW1, iof2, "a1" + s, flush1,
                      skip=("agg1" in SKIP), bufs=3)

            nc.sync.dma_start(T.h1T_loc[:], h1T_sb[:, 0:SLICE])
            nc.gpsimd.collective_compute(
                "AllGather", BYPASS, groups,
                ins=[T.h1T_loc[:].opt()], outs=[T.h1T_ag[:].opt()],
            )
            h1T_full = pp2.tile([128, P, SLICE], F32)
            nc.sync.dma_start(h1T_full[:], T.h1T_ag.ap().rearrange("s f n -> f s n"))
            h1T_flat = h1T_full[:].rearrange("f s n -> f (s n)")

            h2_nm = pp2.tile([128, NBLK, C], F32)
            g2_nm = pp2.tile([128, NBLK, 1], F32)
            with tc.tile_pool(name="ph2" + s, bufs=2, space="PSUM") as ph2p:
                for b in range(NBLK):
                    nv = max(0, min(128, N - b * 128))
                    if nv < 128:
                        nc.vector.memset(h2_nm[:, b, :], 0.0)
                        nc.vector.memset(g2_nm[:, b, :], 0.0)
                    if nv == 0:
                        continue
                    ph2 = ph2p.tile([128, C + 1], F32)
                    nc.tensor.matmul(
                        ph2[0:nv, :],
                        h1T_flat[:, b * 128 : b * 128 + nv],
                        W2cat[:],
                        start=True,
                        stop=True,
                    )
                    nc.vector.tensor_copy(h2_nm[0:nv, b, :], ph2[0:nv, 0:C])
                    nc.scalar.activation(g2_nm[0:nv, b, :], ph2[0:nv, C : C + 1], EXP)

            with tc.tile_pool(name="stage2" + s, bufs=1) as stp:
                st = stp.tile([128, NBLK, 128], F16, tag="stage2")
                nc.vector.memset(st[:], 0.0)
                nc.vector.tensor_copy(st[:, :, 0:1], g2_nm[:])
                nc.sync.dma_start(
                    T.g2_tab.ap().rearrange("(b p) c -> p b c", p=128), st[:]
                )

            with tc.tile_pool(name="zu2" + s, bufs=3) as zup:

                def zflush2(w, po, zup=zup):
                    u_t = zup.tile([128, 1], F32, tag="u2")
                    nc.vector.reciprocal(u_t[:], po[:, 0:1])
                    rows = min(128, SLICE - 128 * w)
                    nc.sync.dma_start(
                        T.u2_sl[w * 128 : w * 128 + rows, :], u_t[0:rows, :]
                    )

                _spmm(nc, tc, zB, 32, T.idx128, 0, rel_sb2, 0, T.g2_tab, 128,
                      8, 8, iof2, "z2" + s, zflush2, skip=("z" in SKIP),
                      bufs=3)

            nc.gpsimd.collective_compute(
                "AllGather", BYPASS, groups,
                ins=[T.u2_sl[:].opt()], outs=[T.u2_full[0:N, :].opt()],
            )
            zt2 = sp2.tile([NPAD - N, 1], F32, tag="zpad2")
            nc.vector.memset(zt2[:], 0.0)
            nc.sync.dma_start(T.u2_full[N:NPAD, :], zt2[:])

            u2_nm = pp2.tile([128, NBLK, 1], F32)
            nc.sync.dma_start(
                u2_nm[:], T.u2_full.ap().rearrange("(b p) c -> p b c", p=128)
            )
            with tc.tile_pool(name="stage3" + s, bufs=1) as stp:
                st = stp.tile([128, NBLK, 128], F16, tag="stage3")
                nc.vector.memset(st[:], 0.0)
                for b in range(NBLK):
                    nc.vector.tensor_scalar(
                        st[:, b, 0:C], h2_nm[:, b, :], u2_nm[:, b, :], None, MULT
                    )
                nc.sync.dma_start(
                    T.hh2_tab.ap().rearrange("(b p) c -> p b c", p=128), st[:]
                )

            with (
                tc.tile_pool(name="gw2" + s, bufs=2) as gwp,
                tc.tile_pool(name="fl2" + s, bufs=2) as flp,
            ):
                gwi = gwp.tile([128, NWIN * 8], I16, tag="gwi2")
                nc.sync.dma_start(
                    gwi[:], T.idx128[:, (ZT + BT) * 8 : (ZT + BT + NWIN) * 8]
                )
                gwb = gwp.tile([128, NWIN, 128], F16, tag="gwb2")
                nc.gpsimd.dma_gather(
                    gwb[:], T.g2_tab[:], gwi[:], NWIN * 128, NWIN * 128, 128,
                    single_packet=False,
                )
                gwf = gwp.tile([128, NWIN, 128], F32, tag="gwf2")
                nc.vector.tensor_copy(gwf[:], gwb[:])

                def flush2(w, po):
                    o2 = flp.tile([128, C], F32, tag="o2")
                    nc.vector.tensor_scalar(
                        o2[:], po[:, 0:C], gwf[:, w, 0:1], None, MULT
                    )
                    rows = min(128, SLICE - 128 * w)
                    nc.sync.dma_start(
                        T.out_d[w * 128 : w * 128 + rows, :], o2[0:rows, :]
                    )

                _spmm(nc, tc, bB, 32, T.idx128, ZT, rel_sb2, ZT, T.hh2_tab,
                      128, C, C, iof2, "a2" + s, flush2,
                      skip=("agg2" in SKIP), bufs=3)


def _build_program(zB, bB, reps=1):
    nc = bacc.Bacc("TRN2", target_bir_lowering=False, debug=False, num_devices=P)
    T = _declare(nc, zB, bB)
    with tile.TileContext(nc) as tc:
        for r in range(reps):
            _emit(nc, tc, T, zB, bB, s=str(r))
            if reps > 1:
                with tc.tile_critical():
                    nc.all_core_barrier()
    nc.compile()
    return nc


def _pack_blobs(x, W1, a1, W2, a2, per_core, zB, bB):
    ZT, BT = sum(zB), sum(bB)
    L = _blob_layout(ZT, BT)
    x = np.asarray(x, np.float32)
    W1 = np.asarray(W1, np.float32)
    a1 = np.asarray(a1, np.float32)
    W2 = np.asarray(W2, np.float32)
    a2 = np.asarray(a2, np.float32)

    xT = np.zeros((F, NPADX), np.float16)
    xT[:, :N] = x.T
    w1_16 = W1.astype(np.float16)                      # [F, OW1]
    prm = np.zeros((128, 28), np.float16)
    prm[0:F, 0:C] = W2
    prm[0:F, 22:26] = a1[:, F : 2 * F].T               # a1rc
    prm[0:C, 26] = a2[0, C : 2 * C]                    # a2rc
    WI8 = L["WI8"]

    blobs = []
    for k in range(P):
        blob = np.zeros((128, L["end"]), np.uint8)
        blob[:, L["x"] : L["x"] + 2 * XS] = np.ascontiguousarray(
            xT[:, k * XS : (k + 1) * XS]
        ).view(np.uint8)
        blob[:, L["w1"] : L["w1"] + 2 * OW1] = w1_16.view(np.uint8)
        blob[:, L["prm"] : L["prm"] + 2 * 28] = prm.view(np.uint8)
        pc = per_core[k]
        blob[:, L["rel"] : L["rel"] + ZT] = pc["zrel"]
        blob[:, L["rel"] + ZT : L["rel"] + ZT + BT] = pc["brel"]
        W16 = np.concatenate([pc["zidx"], pc["bidx"], pc["gwidx"]], axis=1)
        assert W16.shape == (16, WI8 * 8)
        blob[:, L["idx"] : L["idx"] + 2 * WI8] = (
            np.ascontiguousarray(W16.reshape(16, 8, WI8).reshape(128, WI8))
            .view(np.uint8)
        )
        blobs.append(blob)
    return blobs


def build(x, edge_index, W1, a1, W2, a2, reps=1):
    """Build program + per-core input maps. Returns (nc, in_maps)."""
    ei = np.asarray(edge_index)
    row = ei[0].astype(np.int64)
    col = ei[1].astype(np.int64)
    zB, bB, per_core = _build_edge_inputs(row, col)
    nc = _build_program(zB, bB, reps=reps)
    blobs = _pack_blobs(x, W1, a1, W2, a2, per_core, zB, bB)
    in_maps = [{"blob": blobs[k]} for k in range(P)]
    return nc, in_maps


def kernel(x, edge_index, W1, a1, W2, a2):
    nc, in_maps = build(x, edge_index, W1, a1, W2, a2)
    res = run_bass_kernel_spmd(nc, in_maps, list(range(P)))
    return np.concatenate([res.results[k]["out"] for k in range(P)], axis=0)


# revision 3
# speedup vs baseline: 9.3499x; 1.1125x over previous
"""GAT (2-layer) Trainium2 kernel, SPMD across 8 NeuronCores — v2.

Same device algorithm as the baseline (factorized segment softmax +
one-hot-matmul SpMM over dma_gathered table rows), but optimized for
end-to-end wall time per run, which is dominated by host->device input
transfer and per-call jit compilation, not HW execution:

  * ONE packed uint8 input tensor per core (~0.7 MB vs 7.3 MB over 12
    tensors). Sections (per-partition layout, bitcast views on device):
      x shard   f16 [128, XS]    node features, transposed + sharded;
                                 AllGathered on-device across the 8 cores
      W1        f16 [128, 512]
      params    f16 [128, 28]    W2 | a1rc | a2rc
      rel       u8  [128, ZT+BT] block-relative dst indices (0..127)
      idx       i16 [16, WI] wrapped gather indices, stored across 128
                                 partitions; replicated 16->128 on device
  * identity / iota helper matrices generated on device (iota + is_equal)
  * f16 everywhere the baseline used bf16 (same bytes, 8x less rounding)
  * jax persistent compilation cache enabled so repeated
    run_bass_kernel_spmd calls skip the ~1s per-call PJRT re-compile.

kernel(**inputs) takes FULL inputs and returns the FULL [10000, 22] output.
"""

import sys

sys.path.insert(0, "/opt/trn_rl_repo")

import numpy as np

import jax

jax.config.update("jax_compilation_cache_dir", "/tmp/jax_bass_cache")
jax.config.update("jax_persistent_cache_min_entry_size_bytes", -1)
jax.config.update("jax_persistent_cache_min_compile_time_secs", 0)

from concourse import bacc, mybir, tile
from concourse.bass_utils import run_bass_kernel_spmd

F32 = mybir.dt.float32
F16 = mybir.dt.float16
I16 = mybir.dt.int16
I32 = mybir.dt.int32
U8 = mybir.dt.uint8
EXP = mybir.ActivationFunctionType.Exp
EQ = mybir.AluOpType.is_equal
MULT = mybir.AluOpType.mult
ADD = mybir.AluOpType.add
MIN = mybir.AluOpType.min
BYPASS = mybir.AluOpType.bypass

N = 10000
E = 320000
F = 128
H = 4
C = 22
P = 8
SLICE = N // P               # 1250 nodes per core
NWIN = (SLICE + 127) // 128  # 10 windows of <=128 dst/src nodes
NBLK = N // 128 + 1          # 79; always >= 1 pad block so row N is zero
NPAD = NBLK * 128            # 10112; table rows >= N are zero
XBLK = (NBLK + P - 1) // P   # x-shard blocks per core
XS = XBLK * 128              # x-shard width (1280)
NPADX = P * XS               # 10240
OW1 = H * F                  # 512
CHUNK = 16                   # layer-1 gather chunk (128-edge blocks)
SKIP = set()                 # debug/timing: {"z", "agg1", "agg2"}


def _configure(n, e, p=8):
    """Shrink sizes for simulator debugging (same program structure)."""
    global N, E, P, SLICE, NWIN, NBLK, NPAD, XBLK, XS, NPADX
    N, E, P = n, e, p
    SLICE = N // P
    NWIN = (SLICE + 127) // 128
    NBLK = N // 128 + 1
    NPAD = NBLK * 128
    XBLK = (NBLK + P - 1) // P
    XS = XBLK * 128
    NPADX = P * XS


def _cdiv(a, b):
    return (a + b - 1) // b


def _wrap16(idx):
    """dma_gather index layout: logical i at [i%16, i//16] (16 partitions,
    NOT replicated — replication to 128 partitions happens on device)."""
    n = idx.shape[0]
    assert n % 16 == 0
    return idx.reshape(n // 16, 16).T.astype(np.int16)


def _phase_arrays(key, other, nwin):
    """Group one core's (already core-local) edges by 128-wide key window.
    Returns per-window (rel, other) with rel = key - 128*w."""
    w = key >> 7
    order = np.argsort(w, kind="stable")
    key, other, w = key[order], other[order], w[order]
    out = []
    bounds = np.searchsorted(w, np.arange(nwin + 1))
    for i in range(nwin):
        sl = slice(bounds[i], bounds[i + 1])
        k, o = key[sl] - 128 * i, other[sl]
        so = np.argsort(o, kind="stable")  # sorted gather idx -> HBM locality
        out.append((k[so], o[so]))
    return out


def _build_edge_inputs(row, col):
    zraw, braw = [], []
    for k in range(P):
        base = k * SLICE
        m = (row >= base) & (row < base + SLICE)
        zraw.append(_phase_arrays(row[m] - base, col[m], NWIN))
        m = (col >= base) & (col < base + SLICE)
        braw.append(_phase_arrays(col[m] - base, row[m], NWIN))

    def block_counts(raw):
        return [
            max(_cdiv(max(max(len(raw[k][w][0]) for k in range(P)), 1), 128), 1)
            for w in range(NWIN)
        ]

    zB = block_counts(zraw)
    bB = block_counts(braw)

    def pack(raw, B):
        idx_l, rel_l = [], []
        for w in range(NWIN):
            n = B[w] * 128
            rel = np.zeros(n, np.uint8)
            oth = np.full(n, N, np.int32)  # dummy -> zero table row
            r, o = raw[w]
            rel[: len(r)] = r
            oth[: len(o)] = o
            idx_l.append(_wrap16(oth))
            rel_l.append(rel.reshape(B[w], 128).T)
        return np.concatenate(idx_l, 1), np.concatenate(rel_l, 1)

    per_core = []
    for k in range(P):
        zidx, zrel = pack(zraw[k], zB)
        bidx, brel = pack(braw[k], bB)
        base = k * SLICE
        gw = []
        for w in range(NWIN):
            nid = base + 128 * w + np.arange(128)
            nid = np.where(nid < base + SLICE, nid, N)
            gw.append(_wrap16(nid))
        per_core.append(
            dict(zidx=zidx, zrel=zrel, bidx=bidx, brel=brel,
                 gwidx=np.concatenate(gw, 1))
        )
    return zB, bB, per_core


# ---------------------------------------------------------------------------
# blob layout (per-partition byte offsets)


def _blob_layout(ZT, BT):
    WI8 = ZT + BT + NWIN              # i16 per partition of idx section
    WS = OW1 // P                     # W1 shard columns per core (64)
    o = {}
    o["x"] = 0
    o["w1"] = o["x"] + 2 * XS
    o["prm"] = o["w1"] + 2 * WS
    o["rel"] = o["prm"] + 2 * 28
    oidx = o["rel"] + (ZT + BT)
    o["idx"] = oidx + (oidx % 2)      # 2-byte align for i16 bitcast
    o["end"] = o["idx"] + 2 * WI8
    o["WI8"] = WI8
    o["WS"] = WS
    return o


def _spmm(nc, tc, B, CH, idx128, idx_base, rel_sb, rel_base, tab, elem,
          rhs_w, psum_w, iof_t, name, flush, skip=False, bufs=3):
    """One-hot-matmul SpMM over 128-dst windows with gather chunks that span
    window boundaries. flush(w, po) consumes each window's PSUM result.
    idx_base/rel_base are in 128-edge-block units into idx128/rel_sb."""
    with (
        tc.tile_pool(name=f"gg{name}", bufs=bufs) as ggp,
        tc.tile_pool(name=f"gi{name}", bufs=bufs) as gip,
        tc.tile_pool(name=f"go{name}", bufs=bufs) as ohp,
        tc.tile_pool(name=f"gp{name}", bufs=2, space="PSUM") as pp,
    ):
        total = sum(B)
        gts, ohs = {}, {}
        gb = 0
        for w, Bw in enumerate(B):
            po = pp.tile([128, psum_w], F32, tag="po")
            if skip:
                nc.vector.memset(po[:], 1.0)
                flush(w, po)
                continue
            for b in range(Bw):
                ch, off = divmod(gb, CH)
                if off == 0:
                    cb = min(CH, total - ch * CH)
                    it = gip.tile([128, CH * 8], I16, tag="gi")
                    nc.sync.dma_start(
                        it[:, : cb * 8],
                        idx128[:, (idx_base + ch * CH) * 8
                               : (idx_base + ch * CH + cb) * 8],
                    )
                    gt = ggp.tile([128, CH, elem], F16, tag="gg")
                    nc.gpsimd.dma_gather(
                        gt[:, :cb, :], tab[:], it[:, : cb * 8],
                        cb * 128, cb * 128, elem, single_packet=False,
                    )
                    oh = ohp.tile([128, CH, 128], F16, tag="go")
                    nc.vector.tensor_tensor(
                        oh[:, :cb, :],
                        iof_t[:].rearrange("p (x f) -> p x f", x=1)
                        .broadcast_to([128, cb, 128]),
                        rel_sb[:, rel_base + ch * CH : rel_base + ch * CH + cb]
                        .rearrange("p (b x) -> p b x", x=1)
                        .broadcast_to([128, cb, 128]),
                        EQ,
                    )
                    gts[ch], ohs[ch] = gt, oh
                nc.tensor.matmul(
                    po[:], ohs[ch][:, off, :], gts[ch][:, off, 0:rhs_w],
                    start=(b == 0), stop=(b == Bw - 1),
                )
                gb += 1
            flush(w, po)


def _declare(nc, zB, bB):
    ZT, BT = sum(zB), sum(bB)
    L = _blob_layout(ZT, BT)
    T = type("T", (), {})()
    T.L = L
    T.blob = nc.dram_tensor("blob", [128, L["end"]], U8, kind="ExternalInput")
    T.out_d = nc.dram_tensor("out", [SLICE, C], F16, kind="ExternalOutput")

    T.x_sl = nc.dram_tensor("x_sl", [128, XS], F16)
    T.x_ag = nc.dram_tensor("x_ag", [P, 128, XS], F16, addr_space="Shared")
    T.w1_sl = nc.dram_tensor("w1_sl", [128, L["WS"]], F16)
    T.w1_ag = nc.dram_tensor("w1_ag", [P, 128, L["WS"]], F16, addr_space="Shared")
    T.idx128 = nc.dram_tensor("idx128", [128, L["WI8"] * 8], I16)

    T.g1_tab = nc.dram_tensor("g1_tab", [NPAD, 128], F16)
    T.hh1_tab = nc.dram_tensor("hh1_tab", [NPAD, OW1], F16)
    T.g2_tab = nc.dram_tensor("g2_tab", [NPAD, 128], F16)
    T.hh2_tab = nc.dram_tensor("hh2_tab", [NPAD, 128], F16)
    T.u1_sl = nc.dram_tensor("u1_sl", [SLICE, H], F32)
    T.u2_sl = nc.dram_tensor("u2_sl", [SLICE, 1], F32)
    T.u1_full = nc.dram_tensor("u1_full", [NPAD, H], F32, addr_space="Shared")
    T.u2_full = nc.dram_tensor("u2_full", [NPAD, 1], F32, addr_space="Shared")
    T.h1T_loc = nc.dram_tensor("h1T_loc", [F, SLICE], F32)
    T.h1T_ag = nc.dram_tensor("h1T_ag", [P, F, SLICE], F32, addr_space="Shared")

    return T


def _emit(nc, tc, T, zB, bB, s=""):
        groups = [list(range(P))]
        L = T.L
        ZT, BT = sum(zB), sum(bB)
        blob = T.blob.ap()
        xv = blob[:, L["x"] : L["x"] + 2 * XS].bitcast(F16)
        w1v = blob[:, L["w1"] : L["w1"] + 2 * L["WS"]].bitcast(F16)
        prmv = blob[:, L["prm"] : L["prm"] + 2 * 28].bitcast(F16)
        relv = blob[:, L["rel"] : L["rel"] + ZT + BT]
        idxv = (
            blob[:, L["idx"] : L["idx"] + 2 * L["WI8"]]
            .bitcast(I16)
            .rearrange("(a b) c -> a b c", a=16)
        )  # [16, 8, WI8]; element (a,b,c) = W16[a, b*WI8+c]
        RELW = ZT + BT

        # ================= layer 1: dense + tables + z1 =================
        with (
            tc.tile_pool(name="persist" + s, bufs=1) as pp,
            tc.tile_pool(name="small" + s, bufs=3) as sp,
        ):
            # ---- unpack blob: x/W1 shards -> AllGather; idx 16->128 ----
            nc.sync.dma_start(T.x_sl[:], xv)
            nc.gpsimd.collective_compute(
                "AllGather", BYPASS, groups,
                ins=[T.x_sl[:].opt()], outs=[T.x_ag[:].opt()],
            )
            nc.sync.dma_start(T.w1_sl[:], w1v)
            nc.gpsimd.collective_compute(
                "AllGather", BYPASS, groups,
                ins=[T.w1_sl[:].opt()], outs=[T.w1_ag[:].opt()],
            )
            for r in range(8):
                nc.sync.dma_start(
                    T.idx128[16 * r : 16 * r + 16, :]
                    .rearrange("a (b c) -> a b c", b=8),
                    idxv,
                )

            # ---- helper matrices generated on device ----
            it32 = sp.tile([128, 128], I32, tag="it32")
            nc.gpsimd.iota(it32[:], [[1, 128]], base=0, channel_multiplier=0)
            ip32 = sp.tile([128, 128], I32, tag="ip32")
            nc.gpsimd.iota(ip32[:], [[0, 128]], base=0, channel_multiplier=1)
            iof_t = pp.tile([128, 128], F32)
            nc.vector.tensor_copy(iof_t[:], it32[:])
            id16 = pp.tile([128, 128], F16)
            nc.vector.tensor_tensor(id16[:], it32[:], ip32[:], EQ)
            id32 = pp.tile([128, 128], F32)
            nc.vector.tensor_tensor(id32[:], it32[:], ip32[:], EQ)

            # ---- params / weights / rel ----
            W1_t = pp.tile([F, OW1], F16)
            nc.sync.dma_start(
                W1_t[:].rearrange("f (p c) -> f p c", p=P),
                T.w1_ag.ap().rearrange("p f c -> f p c"),
            )
            prm_t = pp.tile([128, 28], F16)
            nc.sync.dma_start(prm_t[:], prmv)
            rel8 = sp.tile([128, RELW], U8, tag="rel8")
            nc.sync.dma_start(rel8[:], relv)
            rel_sb = pp.tile([128, RELW], F32)
            nc.vector.tensor_copy(rel_sb[:], rel8[:])

            W1ar_t = pp.tile([F, H], F16)
            with tc.tile_pool(name="ptr" + s, bufs=2, space="PSUM") as ptr:
                for hd in range(H):
                    pt = ptr.tile([128, 128], F16, tag="pt")
                    nc.tensor.transpose(pt[:], W1_t[:, hd * F : (hd + 1) * F], id16[:])
                    w1t = sp.tile([128, 128], F16, tag="w1t")
                    nc.vector.tensor_copy(w1t[:], pt[:])
                    pv = ptr.tile([128, 1], F32, tag="pv")
                    nc.tensor.matmul(
                        pv[:], w1t[:], prm_t[:, 22 + hd : 23 + hd],
                        start=True, stop=True,
                    )
                    nc.vector.tensor_copy(W1ar_t[:, hd : hd + 1], pv[:])

            xfull = pp.tile([128, P, XS], F16)  # full gathered x^T in SBUF
            nc.sync.dma_start(xfull[:], T.x_ag.ap().rearrange("p f s -> f p s"))

            h_nm = pp.tile([128, NBLK, OW1], F16)  # 10.1 MB
            g1_nm = pp.tile([128, NBLK, H], F32)
            with (
                tc.tile_pool(name="ph" + s, bufs=2, space="PSUM") as php,
                tc.tile_pool(name="psr" + s, bufs=2, space="PSUM") as psrp,
            ):
                for b in range(NBLK):
                    xt = xfull[:, b // XBLK, (b % XBLK) * 128 : (b % XBLK + 1) * 128]
                    ph = php.tile([128, OW1], F32)
                    nc.tensor.matmul(ph[:], xt, W1_t[:], start=True, stop=True)
                    psr = psrp.tile([128, H], F32)
                    nc.tensor.matmul(psr[:], xt, W1ar_t[:], start=True, stop=True)
                    nc.vector.tensor_copy(h_nm[:, b, :], ph[:])
                    nc.scalar.activation(g1_nm[:, b, :], psr[:], EXP)

            with tc.tile_pool(name="stage" + s, bufs=1) as stp:
                st = stp.tile([128, NBLK, 128], F16, tag="stage")
                nc.vector.memset(st[:], 0.0)
                nc.vector.tensor_copy(
                    st[:, : NBLK - 1, 0:H], g1_nm[:, : NBLK - 1, :]
                )
                nv = N - 128 * (NBLK - 1)
                if nv > 0:
                    nc.vector.tensor_copy(
                        st[0:nv, NBLK - 1, 0:H], g1_nm[0:nv, NBLK - 1, :]
                    )
                nc.sync.dma_start(
                    T.g1_tab.ap().rearrange("(b p) c -> p b c", p=128), st[:]
                )

            with tc.tile_pool(name="zu1" + s, bufs=3) as zup:

                def zflush1(w, po, zup=zup):
                    u_t = zup.tile([128, H], F32, tag="u")
                    nc.vector.reciprocal(u_t[:], po[:, 0:H])
                    rows = min(128, SLICE - 128 * w)
                    nc.sync.dma_start(
                        T.u1_sl[w * 128 : w * 128 + rows, :], u_t[0:rows, :]
                    )

                _spmm(nc, tc, zB, 32, T.idx128, 0, rel_sb, 0, T.g1_tab, 128,
                      8, 8, iof_t, "z1" + s, zflush1, skip=("z" in SKIP),
                      bufs=2)

            nc.gpsimd.collective_compute(
                "AllGather", BYPASS, groups,
                ins=[T.u1_sl[:].opt()], outs=[T.u1_full[0:N, :].opt()],
            )
            zt = sp.tile([NPAD - N, H], F32, tag="zpad")
            nc.vector.memset(zt[:], 0.0)
            nc.sync.dma_start(T.u1_full[N:NPAD, :], zt[:])

            u1_nm = pp.tile([128, NBLK, H], F32)
            nc.sync.dma_start(
                u1_nm[:], T.u1_full.ap().rearrange("(b p) c -> p b c", p=128)
            )
            with tc.tile_pool(name="hhp" + s, bufs=3) as hhp:
                for b in range(NBLK):
                    hh = hhp.tile([128, OW1], F16)
                    for hd in range(H):
                        nc.vector.tensor_scalar(
                            hh[:, hd * F : (hd + 1) * F],
                            h_nm[:, b, hd * F : (hd + 1) * F],
                            u1_nm[:, b, hd : hd + 1],
                            None,
                            MULT,
                        )
                    nc.sync.dma_start(
                        T.hh1_tab.ap().rearrange("(b p) c -> p b c", p=128)[:, b, :],
                        hh[:],
                    )

        # ============ layer 1 aggregation + layer 2 (h_nm freed) ============
        with (
            tc.tile_pool(name="persist2" + s, bufs=1) as pp2,
            tc.tile_pool(name="small2" + s, bufs=3) as sp2,
        ):
            ZT, BT = sum(zB), sum(bB)
            it32b = sp2.tile([128, 128], I32, tag="it32b")
            nc.gpsimd.iota(it32b[:], [[1, 128]], base=0, channel_multiplier=0)
            ip32b = sp2.tile([128, 128], I32, tag="ip32b")
            nc.gpsimd.iota(ip32b[:], [[0, 128]], base=0, channel_multiplier=1)
            iof2 = pp2.tile([128, 128], F32)
            nc.vector.tensor_copy(iof2[:], it32b[:])
            id2 = pp2.tile([128, 128], F32)
            nc.vector.tensor_tensor(id2[:], it32b[:], ip32b[:], EQ)
            prm2_t = pp2.tile([128, 28], F16)
            nc.sync.dma_start(prm2_t[:], prmv)
            rel8b = sp2.tile([128, RELW], U8, tag="rel8b")
            nc.sync.dma_start(rel8b[:], relv)
            rel_sb2 = pp2.tile([128, RELW], F32)
            nc.vector.tensor_copy(rel_sb2[:], rel8b[:])

            W2cat = pp2.tile([F, C + 1], F32)
            nc.vector.tensor_copy(W2cat[:, 0:C], prm2_t[:, 0:22])
            with tc.tile_pool(name="ptr2" + s, bufs=2, space="PSUM") as ptr:
                a2rc_t = sp2.tile([F, 1], F32, tag="a2rc")
                nc.vector.tensor_copy(a2rc_t[:], prm2_t[:, 26:27])
                pt = ptr.tile([128, 128], F32, tag="pt2")
                nc.tensor.transpose(pt[0:C, :], W2cat[:, 0:C], id2[:])
                w2t = sp2.tile([128, 128], F32, tag="w2t")
                nc.vector.tensor_copy(w2t[0:C, :], pt[0:C, :])
                pv = ptr.tile([128, 1], F32, tag="pv2")
                nc.tensor.matmul(
                    pv[:], w2t[0:C, :], a2rc_t[0:C, :], start=True, stop=True
                )
                nc.vector.tensor_copy(W2cat[:, C : C + 1], pv[:])

            h1T_sb = pp2.tile([128, NWIN * 128], F32)

            with (
                tc.tile_pool(name="gwp" + s, bufs=2) as gwp,
                tc.tile_pool(name="ptw" + s, bufs=2, space="PSUM") as ptw,
                tc.tile_pool(name="flush" + s, bufs=2) as flp,
            ):
                gwi = gwp.tile([128, NWIN * 8], I16, tag="gwi")
                nc.sync.dma_start(
                    gwi[:], T.idx128[:, (ZT + BT) * 8 : (ZT + BT + NWIN) * 8]
                )
                gwb = gwp.tile([128, NWIN, 128], F16, tag="gwb")
                nc.gpsimd.dma_gather(
                    gwb[:], T.g1_tab[:], gwi[:], NWIN * 128, NWIN * 128, 128,
                    single_packet=False,
                )
                gwf = gwp.tile([128, NWIN, 128], F32, tag="gwf")
                nc.vector.tensor_copy(gwf[:], gwb[:])

                def flush1(w, po):
                    o_t = flp.tile([128, OW1], F32, tag="o")
                    for hd in range(H):
                        nc.vector.tensor_scalar(
                            o_t[:, hd * F : (hd + 1) * F],
                            po[:, hd * F : (hd + 1) * F],
                            gwf[:, w, hd : hd + 1],
                            None, MULT,
                        )
                    # elu(x) = relu(x) + exp(min(x,0)) - 1 ; h1 = mean_heads
                    neg = flp.tile([128, OW1], F32, tag="neg")
                    nc.vector.tensor_scalar(neg[:], o_t[:], 0.0, None, MIN)
                    ex = flp.tile([128, OW1], F32, tag="ex")
                    nc.scalar.activation(ex[:], neg[:], EXP)
                    rl = flp.tile([128, OW1], F32, tag="rl")
                    nc.vector.tensor_relu(rl[:], o_t[:])
                    su = flp.tile([128, OW1], F32, tag="su")
                    nc.vector.tensor_tensor(su[:], rl[:], ex[:], ADD)
                    t01 = flp.tile([128, F], F32, tag="t01")
                    nc.vector.tensor_tensor(t01[:], su[:, 0:F], su[:, F : 2 * F], ADD)
                    t23 = flp.tile([128, F], F32, tag="t23")
                    nc.vector.tensor_tensor(
                        t23[:], su[:, 2 * F : 3 * F], su[:, 3 * F :], ADD
                    )
                    h1_t = flp.tile([128, F], F32, tag="h1")
                    nc.vector.tensor_tensor(h1_t[:], t01[:], t23[:], ADD)
                    nc.vector.tensor_scalar(h1_t[:], h1_t[:], 0.25, -1.0, MULT, ADD)
                    ptt = ptw.tile([128, 128], F32, tag="ptt")
                    nc.tensor.transpose(ptt[:], h1_t[:], id2[:])
                    nc.vector.tensor_copy(h1T_sb[:, w * 128 : (w + 1) * 128], ptt[:])

                _spmm(nc, tc, bB, CHUNK, T.idx128, ZT, rel_sb2, ZT,
                      T.hh1_tab, OW1, OW1, OW1, iof2, "a1" + s, flush1,
                      skip=("agg1" in SKIP), bufs=3)

            nc.sync.dma_start(T.h1T_loc[:], h1T_sb[:, 0:SLICE])
            nc.gpsimd.collective_compute(
                "AllGather", BYPASS, groups,
                ins=[T.h1T_loc[:].opt()], outs=[T.h1T_ag[:].opt()],
            )
            h1T_full = pp2.tile([128, P, SLICE], F32)
            nc.sync.dma_start(h1T_full[:], T.h1T_ag.ap().rearrange("s f n -> f s n"))
            h1T_flat = h1T_full[:].rearrange("f s n -> f (s n)")

            h2_nm = pp2.tile([128, NBLK, C], F32)
            g2_nm = pp2.tile([128, NBLK, 1], F32)
            with tc.tile_pool(name="ph2" + s, bufs=2, space="PSUM") as ph2p:
                for b in range(NBLK):
                    nv = max(0, min(128, N - b * 128))
                    if nv < 128:
                        nc.vector.memset(h2_nm[:, b, :], 0.0)
                        nc.vector.memset(g2_nm[:, b, :], 0.0)
                    if nv == 0:
                        continue
                    ph2 = ph2p.tile([128, C + 1], F32)
                    nc.tensor.matmul(
                        ph2[0:nv, :],
                        h1T_flat[:, b * 128 : b * 128 + nv],
                        W2cat[:],
                        start=True,
                        stop=True,
                    )
                    nc.vector.tensor_copy(h2_nm[0:nv, b, :], ph2[0:nv, 0:C])
                    nc.scalar.activation(g2_nm[0:nv, b, :], ph2[0:nv, C : C + 1], EXP)

            with tc.tile_pool(name="stage2" + s, bufs=1) as stp:
                st = stp.tile([128, NBLK, 128], F16, tag="stage2")
                nc.vector.memset(st[:], 0.0)
                nc.vector.tensor_copy(st[:, :, 0:1], g2_nm[:])
                nc.sync.dma_start(
                    T.g2_tab.ap().rearrange("(b p) c -> p b c", p=128), st[:]
                )

            with tc.tile_pool(name="zu2" + s, bufs=3) as zup:

                def zflush2(w, po, zup=zup):
                    u_t = zup.tile([128, 1], F32, tag="u2")
                    nc.vector.reciprocal(u_t[:], po[:, 0:1])
                    rows = min(128, SLICE - 128 * w)
                    nc.sync.dma_start(
                        T.u2_sl[w * 128 : w * 128 + rows, :], u_t[0:rows, :]
                    )

                _spmm(nc, tc, zB, 32, T.idx128, 0, rel_sb2, 0, T.g2_tab, 128,
                      8, 8, iof2, "z2" + s, zflush2, skip=("z" in SKIP),
                      bufs=3)

            nc.gpsimd.collective_compute(
                "AllGather", BYPASS, groups,
                ins=[T.u2_sl[:].opt()], outs=[T.u2_full[0:N, :].opt()],
            )
            zt2 = sp2.tile([NPAD - N, 1], F32, tag="zpad2")
            nc.vector.memset(zt2[:], 0.0)
            nc.sync.dma_start(T.u2_full[N:NPAD, :], zt2[:])

            u2_nm = pp2.tile([128, NBLK, 1], F32)
            nc.sync.dma_start(
                u2_nm[:], T.u2_full.ap().rearrange("(b p) c -> p b c", p=128)
            )
            with tc.tile_pool(name="stage3" + s, bufs=1) as stp:
                st = stp.tile([128, NBLK, 128], F16, tag="stage3")
                nc.vector.memset(st[:], 0.0)
                for b in range(NBLK):
                    nc.vector.tensor_scalar(
                        st[:, b, 0:C], h2_nm[:, b, :], u2_nm[:, b, :], None, MULT
                    )
                nc.sync.dma_start(
                    T.hh2_tab.ap().rearrange("(b p) c -> p b c", p=128), st[:]
                )

            with (
                tc.tile_pool(name="gw2" + s, bufs=2) as gwp,
                tc.tile_pool(name="fl2" + s, bufs=2) as flp,
            ):
                gwi = gwp.tile([128, NWIN * 8], I16, tag="gwi2")
                nc.sync.dma_start(
                    gwi[:], T.idx128[:, (ZT + BT) * 8 : (ZT + BT + NWIN) * 8]
                )
                gwb = gwp.tile([128, NWIN, 128], F16, tag="gwb2")
                nc.gpsimd.dma_gather(
                    gwb[:], T.g2_tab[:], gwi[:], NWIN * 128, NWIN * 128, 128,
                    single_packet=False,
                )
                gwf = gwp.tile([128, NWIN, 128], F32, tag="gwf2")
                nc.vector.tensor_copy(gwf[:], gwb[:])

                def flush2(w, po):
                    o2 = flp.tile([128, C], F16, tag="o2")
                    nc.vector.tensor_scalar(
                        o2[:], po[:, 0:C], gwf[:, w, 0:1], None, MULT
                    )
                    rows = min(128, SLICE - 128 * w)
                    nc.sync.dma_start(
                        T.out_d[w * 128 : w * 128 + rows, :], o2[0:rows, :]
                    )

                _spmm(nc, tc, bB, 32, T.idx128, ZT, rel_sb2, ZT, T.hh2_tab,
                      128, C, C, iof2, "a2" + s, flush2,
                      skip=("agg2" in SKIP), bufs=3)


def _build_program(zB, bB, reps=1):
    nc = bacc.Bacc("TRN2", target_bir_lowering=False, debug=False, num_devices=P)
    T = _declare(nc, zB, bB)
    with tile.TileContext(nc) as tc:
        for r in range(reps):
            _emit(nc, tc, T, zB, bB, s=str(r))
            if reps > 1:
                with tc.tile_critical():
                    nc.all_core_barrier()
    nc.compile()
    return nc


def _pack_blobs(x, W1, a1, W2, a2, per_core, zB, bB):
    ZT, BT = sum(zB), sum(bB)
    L = _blob_layout(ZT, BT)
    x = np.asarray(x, np.float32)
    W1 = np.asarray(W1, np.float32)
    a1 = np.asarray(a1, np.float32)
    W2 = np.asarray(W2, np.float32)
    a2 = np.asarray(a2, np.float32)

    xT = np.zeros((F, NPADX), np.float16)
    xT[:, :N] = x.T
    w1_16 = W1.astype(np.float16)                      # [F, OW1]
    WS = L["WS"]
    prm = np.zeros((128, 28), np.float16)
    prm[0:F, 0:C] = W2
    prm[0:F, 22:26] = a1[:, F : 2 * F].T               # a1rc
    prm[0:C, 26] = a2[0, C : 2 * C]                    # a2rc
    WI8 = L["WI8"]

    blobs = []
    for k in range(P):
        blob = np.zeros((128, L["end"]), np.uint8)
        blob[:, L["x"] : L["x"] + 2 * XS] = np.ascontiguousarray(
            xT[:, k * XS : (k + 1) * XS]
        ).view(np.uint8)
        blob[:, L["w1"] : L["w1"] + 2 * WS] = np.ascontiguousarray(
            w1_16[:, k * WS : (k + 1) * WS]
        ).view(np.uint8)
        blob[:, L["prm"] : L["prm"] + 2 * 28] = prm.view(np.uint8)
        pc = per_core[k]
        blob[:, L["rel"] : L["rel"] + ZT] = pc["zrel"]
        blob[:, L["rel"] + ZT : L["rel"] + ZT + BT] = pc["brel"]
        W16 = np.concatenate([pc["zidx"], pc["bidx"], pc["gwidx"]], axis=1)
        assert W16.shape == (16, WI8 * 8)
        blob[:, L["idx"] : L["idx"] + 2 * WI8] = (
            np.ascontiguousarray(W16.reshape(16, 8, WI8).reshape(128, WI8))
            .view(np.uint8)
        )
        blobs.append(blob)
    return blobs


def build(x, edge_index, W1, a1, W2, a2, reps=1):
    """Build program + per-core input maps. Returns (nc, in_maps)."""
    ei = np.asarray(edge_index)
    row = ei[0].astype(np.int64)
    col = ei[1].astype(np.int64)
    zB, bB, per_core = _build_edge_inputs(row, col)
    nc = _build_program(zB, bB, reps=reps)
    blobs = _pack_blobs(x, W1, a1, W2, a2, per_core, zB, bB)
    in_maps = [{"blob": blobs[k]} for k in range(P)]
    return nc, in_maps


def kernel(x, edge_index, W1, a1, W2, a2):
    nc, in_maps = build(x, edge_index, W1, a1, W2, a2)
    res = run_bass_kernel_spmd(nc, in_maps, list(range(P)))
    return np.concatenate(
        [res.results[k]["out"] for k in range(P)], axis=0
    ).astype(np.float32)
